# revision 2
# baseline (speedup 1.0000x reference)
"""DenseCRF loss kernel for Trainium2, data-parallel over batch on 8 NeuronCores.

reference:
  seg = bilinear_resize(segmentations, 128->64)            # [N,K,64,64]
  f_i = [x_i/50, y_i/50, r_i/15, g_i/15, b_i/15]           # 5-dim bilateral feature
  W_ij = exp(-0.5*|f_i - f_j|^2)                           # [P,P], P=4096
  loss = WEIGHT * (-sum_k s_k^T W s_k) / N

v2 design (per core, 1 image):
  * Symmetry: only upper-triangle 128-row blocks computed (528/1024 of P^2);
    quadratic form = 2*sum_{r<=c} s_r^T E_rc s_c, diag blocks at half weight
    (0.5-scaled S^T moving operand).
  * G psum = -0.5|f~_i - f~_j|^2 + BETA directly, via a 24-row bf16 Gram
    (f~ = bf16-rounded features; -q rows carried as per-channel -c^2/2 split
    hi/lo so all products are fp32-exact; q_i, q_j and the exp offset BETA
    all folded into the matmul).  bf16 rows halve the DMA cost of the
    [rows, 4096] FA/FB assembly (the v1 cost model charges free-dim bytes
    per DMA regardless of partition count).
  * exp split across ACT (exact Exp, bias=-BETA) and DVE (Schraudolph:
    int16(max(x*128*log2e, 0)) = bf16 bit pattern; SIGMA calibrates the mean).
    Diagonal-containing chunks forced to ACT so W_ii stays exact.
  * Accumulation: E block is the matmul *stationary*, S^T chunk [128,21] the
    moving operand -> 21 cols/block instead of 128.  32 [128,21] acc slices
    packed into two PSUM banks.
  * Finish: two tensor_tensor_reduce against S^T, partition_all_reduce,
    scale by -2*WEIGHT/N; host sums the 8 per-core scalars.
"""

import sys

sys.path.insert(0, "/opt/trn_rl_repo")

import numpy as np
import ml_dtypes

import concourse.bass as bass
import concourse.tile as tile
from concourse import bacc, bass_isa, mybir
from concourse.bass_utils import run_bass_kernel_spmd

F32 = mybir.dt.float32
F32R = mybir.dt.float32r
BF16 = mybir.dt.bfloat16
I16 = mybir.dt.int16
AF = mybir.ActivationFunctionType
ALU = mybir.AluOpType
BF = ml_dtypes.bfloat16

N, C, K = 8, 3, 21
H, W = 64, 64
P = H * W  # 4096
NB = 32  # 128-row blocks
SIGMA_RGB = 15.0
SXY = 100.0 * 0.5
WEIGHT = 1e-8

LN2 = float(np.log(2.0))
LOG2E = float(np.log2(np.e))
SIGMA = 0.0536  # Schraudolph mean-zero calibration
BETA = (127.0 - SIGMA) * LN2
# the value the matmul actually adds: exact-bf16 hi/lo pair
BETA_EFF = float(np.float32(88.0) + np.float32(ml_dtypes.bfloat16(np.float32(BETA) - np.float32(88.0))))
A16 = 128.0 * LOG2E
DVE_RESERVE = 0.0

GW = 1024  # G psum tile width (2 banks; 3 bufs + 2 acc banks = 8)
NWARM1, NWARM2 = 52, 0  # PE p-state warmup dummies (pre / post prep)


def _resize_matrix():
    """[64,128] weights of jax.image.resize(..., method='bilinear') along one
    dim (triangle kernel, antialias=True, scale=0.5, renormalized)."""
    y = np.arange(128, dtype=np.float64)[:, None]
    sample = 2.0 * np.arange(64, dtype=np.float64)[None, :] + 0.5
    w = np.maximum(0.0, 1.0 - 0.5 * np.abs(y - sample))
    w = w / w.sum(axis=0, keepdims=True)
    return np.ascontiguousarray(w.T.astype(np.float32))  # [64,128]


def _schedule():
    """Static per-row chunk schedule: [(col_base, width, engine)].

    All chunks are 512 wide (one PSUM bank).  Each exp engine (ACT / DVE /
    Pool-Schraudolph) has its OWN double-buffered G pool, so an engine's next
    G tile only ever waits on its own previous exp.  Assignment is greedy on
    projected engine time; chunk 0 (diagonal block) is forced to ACT.
    """
    t = {"A": 0.0, "D": DVE_RESERVE, "P": POOL_RESERVE}
    cost = {
        "A": lambda w: w * 0.8333 + 185.0,
        "D": lambda w: w * 1.0417 + 125.0,
        "P": lambda w: w * 1.3889 + 160.0,
    }
    sched = {}
    for r in range(NB):
        base = 128 * r
        row = []
        pos = base
        rem = P - base
        first = True
        while rem > 0:
            w = min(512, rem)
            if first:
                e = "A"
                first = False
            else:
                e = min("ADP", key=lambda k: t[k] + cost[k](w))
            t[e] += cost[e](w)
            row.append((pos, w, e))
            pos += w
            rem -= w
        sched[r] = row
    return sched





def _consts():
    R = _resize_matrix()  # [64,128]
    rtf = np.ascontiguousarray(R.T)  # [128,64] f32
    cbm = np.zeros((128, 128), dtype=BF)
    cbm[:, 0:64] = rtf.astype(BF)
    cbm[0:64, 64:128] = np.eye(64, dtype=BF)

    i = np.arange(P, dtype=np.float64)
    px = (i % 64).astype(np.float32) / np.float32(SXY)
    py = (i // 64).astype(np.float32) / np.float32(SXY)
    pxb = px.astype(BF)
    pyb = py.astype(BF)
    # qpos consistent with the bf16-rounded position rows
    qpos = -0.5 * (pxb.astype(np.float64) ** 2 + pyb.astype(np.float64) ** 2)
    ones = np.ones(P, dtype=BF)

    # beta ~ 88 is too coarse for one bf16 row (quantum 0.5); carry it as an
    # exact hi/lo pair of constant rows paired with FA ones.
    bh = np.float32(88.0)
    bl = np.float32(BF(np.float32(BETA) - bh))
    # FA rows: [qposA, 1, 1, 1, px, py | shA(4) slA(4) cA(4) dyn | 1 x8]
    # FB rows: [1, qposB, bh, bl, px, py, 1 x8 | cB(4) shB(4) slB(4) dyn]
    skelFA1 = np.stack(
        [qpos.astype(BF), ones, ones, ones, pxb, pyb]
    )  # [6,P] -> FA[0:6]
    ones8 = np.broadcast_to(ones, (8, P)).copy()  # FA[18:26]
    skelFB = np.stack(
        [ones, qpos.astype(BF), np.full(P, bh, BF), np.full(P, bl, BF), pxb, pyb]
        + [ones] * 8
    )  # [14,P] -> FB[0:14]
    return dict(
        skelFA1=np.ascontiguousarray(skelFA1),
        ones8=np.ascontiguousarray(ones8),
        skelFB=np.ascontiguousarray(skelFB),
        cbm=cbm,
    )


def _build():
    nc = bacc.Bacc()
    images_d = nc.dram_tensor("images", [C, H, W], F32, kind="ExternalInput")
    seg_d = nc.dram_tensor("segmentations", [K, 128, 128], F32, kind="ExternalInput")
    skelFA1_d = nc.dram_tensor("skelFA1", [6, P], BF16, kind="ExternalInput")
    ones8_d = nc.dram_tensor("ones8", [8, P], BF16, kind="ExternalInput")
    skelFB_d = nc.dram_tensor("skelFB", [14, P], BF16, kind="ExternalInput")
    cbm_d = nc.dram_tensor("cbm", [128, 128], BF16, kind="ExternalInput")
    out_d = nc.dram_tensor("out", [1], F32, kind="ExternalOutput")

    sched = _schedule()
    inv15 = float(np.float32(1.0) / np.float32(SIGMA_RGB))
    segv = seg_d.rearrange("k y x -> y k x")

    with tile.TileContext(nc) as tc:
        with tc.tile_pool(name="persist", bufs=1) as pp:
            img24 = pp.tile([24, 512], F32, tag="img24")
            sfull = pp.tile([24, 512], F32, tag="sfull")
            tslA = pp.tile([96, 512], BF16, tag="tslA")  # [sh | sl | c]
            tslB = pp.tile([96, 512], BF16, tag="tslB")  # [c | sh | sl]
            FA = pp.tile([26, P], BF16, tag="FA")
            FB = pp.tile([26, P], BF16, tag="FB")
            cb_s = pp.tile([128, 128], BF16, tag="cb")
            seg_s = pp.tile([128, K * 128], BF16, tag="seg")
            A_sb = pp.tile([64, K * 128], BF16, tag="A_sb")
            At = pp.tile([128, K * 64], BF16, tag="At")
            STt = pp.tile([128, K * NB], BF16, tag="STt")
            STth = pp.tile([128, K * NB], BF16, tag="STth")
            scr = pp.tile([128, 256], BF16, tag="scr")
            biasb = pp.tile([128, 1], F32, tag="biasb")
            sc = pp.tile([128, 336], F32, tag="sc")
            red1 = pp.tile([128, 1], F32, tag="red1")
            red2 = pp.tile([128, 1], F32, tag="red2")
            tot = pp.tile([128, 1], F32, tag="tot")
            osb = pp.tile([1, 1], F32, tag="osb")

            # ---- t0 DMAs.  v1 cost model: a DMA's queue-busy = free-dim
            # bytes x 0.385ns (x2 if min contiguous run < 512B), regardless
            # of partition count — so [*, 4096] bf16 writes cost ~3.15us
            # each and must be consolidated.  Queues: SP, ACT, Pool(SWDGE).
            nc.sync.dma_start(
                img24[:], images_d.rearrange("c (b h8) w -> (c b) (h8 w)", h8=8)
            )
            nc.scalar.dma_start(FB[0:14, :], skelFB_d[:])
            nc.sync.dma_start(FA[18:26, :], ones8_d[:])
            nc.gpsimd.dma_start(cb_s[:], cbm_d[:])
            # gpsimd DMA casts f32->bf16 in flight; bf16 halves the seg cost
            nc.gpsimd.dma_start(seg_s[:], segv[:])
            nc.gpsimd.dma_start(FA[0:6, :], skelFA1_d[:])

            nc.vector.memset(scr[:], 0.0)
            nc.vector.memset(biasb[:], -BETA_EFF)
            nc.vector.memset(tslA[:], 0.0)
            nc.vector.memset(tslB[:], 0.0)

            # ---- staging chain (DVE): c~ = bf16(img/15); s = -c~^2/2 split
            # hi/lo (exact as a bf16 pair).  Block layout: 32 rows per
            # quantity (24 real + 8 zero pad) so each F tensor's dynamic rows
            # are ONE contiguous DMA from one staging tile.
            nc.vector.tensor_scalar(tslB[0:24, :], img24[:], inv15, None, op0=ALU.mult)
            nc.vector.tensor_copy(tslA[64:88, :], tslB[0:24, :])
            nc.vector.scalar_tensor_tensor(
                sfull[:], tslB[0:24, :], -0.5, tslB[0:24, :], ALU.mult, ALU.mult
            )
            nc.vector.tensor_copy(tslA[0:24, :], sfull[:])
            nc.vector.scalar_tensor_tensor(
                tslA[32:56, :], sfull[:], 1.0, tslA[0:24, :], ALU.mult, ALU.subtract
            )
            nc.vector.tensor_copy(tslB[32:56, :], tslA[0:24, :])
            nc.vector.tensor_copy(tslB[64:88, :], tslA[32:56, :])
            # dynamic rows: FA[6:18] = [shA slA cA], FB[14:26] = [cB shB slB]
            nc.sync.dma_start(FA[6:18, :], tslA[:])
            nc.scalar.dma_start(FB[14:26, :], tslB[:])

            with (
                tc.tile_pool(name="gpsA", bufs=2, space="PSUM") as gpsA,
                tc.tile_pool(name="gpsD", bufs=2, space="PSUM") as gpsD,
                tc.tile_pool(name="gpsP", bufs=2, space="PSUM") as gpsP,
                tc.tile_pool(name="ep", bufs=44) as ep,
            ):
                ets = {}  # (r, ci) -> (et tile, col_base, width)

                def emit_g(r):
                    pools = {"A": gpsA, "D": gpsD, "P": gpsP}
                    for ci, (cb0, w, e) in enumerate(sched[r]):
                        gt = pools[e].tile(
                            [128, 512], F32, tag="g" + e, name=f"g{r}_{ci}"
                        )
                        nc.tensor.matmul(
                            gt[:, 0:w],
                            FA[:, 128 * r : 128 * r + 128],
                            FB[:, cb0 : cb0 + w],
                            start=True,
                            stop=True,
                        )
                        et = ep.tile([128, 512], BF16, tag="e", name=f"e{r}_{ci}")
                        if e == "A":
                            nc.scalar.activation(
                                et[:, 0:w], gt[:, 0:w], AF.Exp, bias=biasb[:]
                            )
                        else:
                            eng = nc.vector if e == "D" else nc.gpsimd
                            eng.tensor_scalar(
                                et[:, 0:w].bitcast(I16),
                                gt[:, 0:w],
                                A16,
                                0.0,
                                op0=ALU.mult,
                                op1=ALU.max,
                            )
                        ets[(r, ci)] = (et, cb0, w)

                def emit_acc(r, accA, accB):
                    for ci in range(len(sched[r])):
                        et, cb0, w = ets.pop((r, ci))
                        for j in range(w // 128):
                            c = (cb0 + 128 * j) // 128
                            mov = STth if c == r else STt
                            acc = accA if c < 16 else accB
                            s = c % 16
                            nc.tensor.matmul(
                                acc[:, 21 * s : 21 * s + 21],
                                et[:, 128 * j : 128 * j + 128],
                                mov[:, 21 * r : 21 * r + 21],
                                start=(r == 0 and s == 0),
                                stop=(r == c == 15 or r == c == 31),
                                skip_group_check=True,
                            )

                with tc.tile_pool(name="pps", bufs=2, space="PSUM") as pps:
                    # PE p-state warmup while FA/FB DMAs are in flight
                    for wi in range(NWARM1):
                        wps = pps.tile([128, 512], F32, tag="p", name=f"w{wi}")
                        nc.tensor.matmul(
                            wps[:, 0:256], scr[:, 0:128], scr[:, 0:256],
                            start=True, stop=True,
                        )
                    emit_g(0)
                    emit_g(1)
                    emit_g(2)
                    emit_g(3)

                    # ---- seg resize prep (PE matmuls + Pool copies)
                    for o in range(0, K * 128, 512):
                        o1 = min(o + 512, K * 128)
                        aps = pps.tile([64, 512], F32, tag="p", name=f"aps{o}")
                        nc.tensor.matmul(
                            aps[:, : o1 - o],
                            cb_s[:, 0:64],
                            seg_s[:, o:o1],
                            start=True,
                            stop=True,
                        )
                        nc.gpsimd.tensor_copy(A_sb[:, o:o1], aps[:, : o1 - o])
                    for k0 in range(0, K, 8):
                        k1 = min(k0 + 8, K)
                        tps = pps.tile([128, 64 * 8], BF16, tag="p", name=f"tps{k0}")
                        for k in range(k0, k1):
                            nc.tensor.transpose(
                                tps[:, 64 * (k - k0) : 64 * (k - k0 + 1)],
                                A_sb[0:64, 128 * k : 128 * (k + 1)],
                                cb_s[0:64, 64:128],
                            )
                        nc.gpsimd.tensor_copy(
                            At[:, 64 * k0 : 64 * k1], tps[:, : 64 * (k1 - k0)]
                        )
                    at3 = At[:].rearrange("x (k y) -> x k y", k=K, y=64)
                    for g in range(4):
                        stp = pps.tile([64, 336], F32, tag="p", name=f"stp{g}")
                        for cl in range(8):
                            cc = 8 * g + cl
                            nc.tensor.matmul(
                                stp[:, 42 * cl : 42 * cl + 42],
                                cb_s[:, 0:64],
                                at3[:, :, 2 * cc : 2 * cc + 2],
                                start=True,
                                stop=True,
                            )
                        st3 = stp[:].rearrange("x (c k y) -> x c k y", c=8, y=2)
                        for yl in range(2):
                            nc.gpsimd.tensor_copy(
                                STt[64 * yl : 64 * yl + 64, 168 * g : 168 * g + 168],
                                st3[:, :, :, yl],
                            )
                    nc.gpsimd.tensor_scalar(STth[:], STt[:], 0.5, None, op0=ALU.mult)


                with tc.tile_pool(name="accps", bufs=1, space="PSUM") as acp:
                    accA = acp.tile([128, 336], F32, tag="accA")
                    accB = acp.tile([128, 336], F32, tag="accB")
                    for r in range(4, NB):
                        emit_g(r)
                        emit_acc(r - 4, accA, accB)
                        if r - 4 == 15:
                            # bank A complete: fold it while the loop runs
                            nc.vector.tensor_tensor_reduce(
                                sc[:], accA[:], STt[:, 0:336], 1.0, 0.0,
                                ALU.mult, ALU.add, red1[:],
                            )
                    emit_acc(NB - 4, accA, accB)
                    emit_acc(NB - 3, accA, accB)
                    emit_acc(NB - 2, accA, accB)
                    emit_acc(NB - 1, accA, accB)
                    nc.vector.tensor_tensor_reduce(
                        sc[:], accB[:], STt[:, 336:672], 1.0, red1[:], ALU.mult,
                        ALU.add, red2[:],
                    )
            nc.gpsimd.partition_all_reduce(tot[:], red2[:], 128, bass_isa.ReduceOp.add)
            nc.vector.tensor_scalar(
                osb[:], tot[0:1, :], float(-2.0 * WEIGHT / N), None, op0=ALU.mult
            )
            nc.sync.dma_start(out_d[:], osb[:])

    nc.finalize()
    return nc


_CACHE = {}


def _get_nc():
    if "nc" not in _CACHE:
        _CACHE["nc"] = _build()
    return _CACHE["nc"]


def kernel(images: np.ndarray, segmentations: np.ndarray) -> np.ndarray:
    images = np.ascontiguousarray(np.asarray(images, dtype=np.float32))
    segmentations = np.ascontiguousarray(np.asarray(segmentations, dtype=np.float32))
    assert images.shape == (N, C, H, W) and segmentations.shape == (N, K, 128, 128)
    nc = _get_nc()
    consts = _consts()
    in_maps = [
        {"images": images[n], "segmentations": segmentations[n], **consts}
        for n in range(N)
    ]
    res = run_bass_kernel_spmd(nc, in_maps, list(range(N)))
    total = sum(float(res.results[n]["out"][0]) for n in range(N))
    return np.array([total], dtype=np.float32)


if __name__ == "__main__":
    rng = np.random.RandomState(0)
    img = rng.rand(N, C, H, W).astype(np.float32) * 255.0
    seg = rng.rand(N, K, 128, 128).astype(np.float32)
    print(kernel(img, seg))
